# revision 28
# baseline (speedup 1.0000x reference)
import sys, time
from collections import deque
from functools import partial

sys.path.insert(0, "/opt/trn_rl_repo")
import numpy as np
import ml_dtypes
from concourse import bass, bacc, tile, mybir
from concourse.bass_utils import run_bass_kernel_spmd

F32 = mybir.dt.float32
BF16 = mybir.dt.bfloat16
I32 = mybir.dt.int32

B, N, DIM = 4, 2048, 1024
HEADS, DH = 16, 64
G = 8          # heads per core
GI = G * DH    # 512 = inner width per core
SCALE = DH ** -0.5
LOG2E = 1.4426950408889634
LN2 = 0.6931471805599453
NB = N // 128   # 16 j-blocks
NCH = N // 512  # 4 q-chunks
DT = DIM // 128  # 8 dim tiles
IT = GI // 128   # 4 inner tiles (= head pairs)

PV_DT = BF16

_CACHE = {}


def _build():
    nc = bacc.Bacc(None, target_bir_lowering=False)
    xT = nc.declare_dram_parameter("xT", [DIM, N], BF16, isOutput=False)
    wq = nc.declare_dram_parameter("wq", [DIM, GI], BF16, isOutput=False)
    wk = nc.declare_dram_parameter("wk", [DIM, GI], BF16, isOutput=False)
    wv = nc.declare_dram_parameter("wv", [DIM, GI], BF16, isOutput=False)
    wo = nc.declare_dram_parameter("wo", [GI, DIM], BF16, isOutput=False)
    msk = nc.declare_dram_parameter("msk", [128, 128], PV_DT, isOutput=False)
    onesf = nc.declare_dram_parameter("onesf", [128, 64], BF16, isOutput=False)
    out = nc.declare_dram_parameter("out", [N, DIM], BF16, isOutput=True)

    EXPF = mybir.ActivationFunctionType.Exp
    AL = mybir.AluOpType

    with tile.TileContext(nc) as tc:
        with (
            nc.allow_low_precision(reason="attention P/V in bf16; rel-err gate 2e-2"),
            tc.tile_pool(name="big", bufs=1) as big,
            tc.tile_pool(name="pt", bufs=6) as ptp,
            tc.tile_pool(name="st", bufs=3) as stp,
            tc.tile_pool(name="rc", bufs=3) as rcp,
            tc.tile_pool(name="raw", bufs=4) as rawp,
            tc.tile_pool(name="on", bufs=4) as onp,
            tc.tile_pool(name="psS", bufs=2, space="PSUM") as psS,
            tc.tile_pool(name="psPo", bufs=2, space="PSUM") as psPo,
            tc.tile_pool(name="psM", bufs=2, space="PSUM") as psM,
        ):
            # ---- persistent SBUF ----
            xTall = big.tile([128, DT * N], BF16, name="xTall", tag="xTall")
            wqall = big.tile([128, DT * GI], BF16, name="wqall", tag="wqall")
            wkall = big.tile([128, DT * GI], BF16, name="wkall", tag="wkall")
            wvall = big.tile([128, DT * GI], BF16, name="wvall", tag="wvall")
            woall = big.tile([128, IT * DIM], BF16, name="woall", tag="woall")
            # per-chunk q/k tiles: [ti][ch] -> [128, 512]
            qTc = [[big.tile([128, 512], BF16, name=f"q{i}_{c}", tag=f"q{i}_{c}") for c in range(NCH)]
                   for i in range(IT)]
            kTc = [[big.tile([128, 512], BF16, name=f"k{i}_{c}", tag=f"k{i}_{c}") for c in range(NCH)]
                   for i in range(IT)]
            # v with ones col per head: [128, 8*65] per j-block
            vg = [big.tile([128, G * (DH + 1)], PV_DT, name=f"v{r}", tag=f"v{r}") for r in range(NB)]
            mask = big.tile([128, 128], PV_DT, name="mask", tag="mask")
            ones1 = big.tile([128, 64], BF16, name="ones1", tag="ones1")
            # ot tiles: one generation per chunk (no cross-chunk hazards)
            otg = [[big.tile([128, 512], BF16, name=f"ot{c2}_{i}", tag=f"ot{c2}_{i}") for i in range(IT)]
                   for c2 in range(NCH)]

            # ---- input DMAs: few big strided transfers (issue cost ~600ns
            # each on the initiating engine, so fewer is much better) ----
            def _x_dst(c):
                return xTall[:].rearrange("p (d n) -> p d n", n=N)[:, :, c * 512:(c + 1) * 512]

            def _x_src(c):
                return xT.rearrange("(d p) n -> p d n", p=128)[:, :, c * 512:(c + 1) * 512]

            # x chunk 0 split across the two HW DMA queues so the first
            # k-projection can start as early as possible
            def _x_dst_h(c, h):
                return xTall[:].rearrange("p (d n) -> p d n", n=N)[
                    :, h * 4:(h + 1) * 4, c * 512:(c + 1) * 512]

            def _x_src_h(c, h):
                return xT.rearrange("(d p) n -> p d n", p=128)[
                    :, h * 4:(h + 1) * 4, c * 512:(c + 1) * 512]

            # weights split in half across both HW queues, ordered by first
            # use: wk (k0 proj) -> wq -> wv -> mask; late x chunks go via the
            # gpsimd (SWDGE) queue since they aren't needed until later
            def _w_half(dst, src, h):
                return (dst[:].rearrange("p (d c) -> p d c", c=GI)[:, h * 4:(h + 1) * 4, :],
                        src.rearrange("(d p) c -> p d c", p=128)[:, h * 4:(h + 1) * 4, :])

            nc.sync.dma_start(_x_dst_h(0, 0), _x_src_h(0, 0))
            nc.scalar.dma_start(_x_dst_h(0, 1), _x_src_h(0, 1))
            nc.sync.dma_start(*_w_half(wkall, wk, 0))
            nc.scalar.dma_start(*_w_half(wkall, wk, 1))
            nc.sync.dma_start(*_w_half(wqall, wq, 0))
            nc.scalar.dma_start(*_w_half(wqall, wq, 1))
            nc.sync.dma_start(*_w_half(wvall, wv, 0))
            nc.scalar.dma_start(*_w_half(wvall, wv, 1))
            nc.scalar.dma_start(mask[:], msk[:])
            nc.scalar.dma_start(ones1[:], onesf[:])
            wupS = big.tile([1, 64], BF16, name="wup", tag="wup")
            nc.gpsimd.memset(wupS[:], 1.0)
            for r in range(NB):
                dst = vg[r][:].rearrange("p (h c) -> p h c", c=DH + 1)[:, :, DH:DH + 1]
                nc.gpsimd.memset(dst, 1.0)
            nc.sync.dma_start(_x_dst(1), _x_src(1))
            nc.scalar.dma_start(woall[:].rearrange("p (i c) -> p i c", c=DIM),
                                wo.rearrange("(i p) c -> p i c", p=128))
            nc.gpsimd.dma_start(_x_dst(2), _x_src(2))
            nc.gpsimd.dma_start(_x_dst(3), _x_src(3))

            # ---- filler machinery ----
            class Filler:
                def __init__(self):
                    self.q = deque()

                def add(self, thunks):
                    self.q.extend(thunks)

                def drain(self, n):
                    for _ in range(n):
                        if not self.q:
                            return
                        self.q.popleft()()

                def drain_all(self):
                    while self.q:
                        self.q.popleft()()

            # ---- projection emission (as thunks) ----
            def proj_mm(kind, g, c, pq, d0, d1):
                for d in range(d0, d1):
                    if kind == "q":
                        nc.tensor.matmul(
                            pq[:],
                            wqall[:, d * GI + g * 128:d * GI + (g + 1) * 128],
                            xTall[:, d * N + c * 512:d * N + (c + 1) * 512],
                            start=(d == 0), stop=(d == DT - 1))
                    elif kind == "k":
                        nc.tensor.matmul(
                            pq[:],
                            wkall[:, d * GI + g * 128:d * GI + (g + 1) * 128],
                            xTall[:, d * N + c * 512:d * N + (c + 1) * 512],
                            start=(d == 0), stop=(d == DT - 1))
                    else:  # v: rows block r = 4c+g
                        r = 4 * c + g
                        nc.tensor.matmul(
                            pq[:],
                            xTall[:, d * N + r * 128:d * N + (r + 1) * 128],
                            wvall[:, d * GI:(d + 1) * GI],
                            start=(d == 0), stop=(d == DT - 1))

            def proj_evac(kind, g, c, pq):
                # chunks 0-2 drain while the scalar engine is mostly idle:
                # evacuating there unblocks psM slot recycling for the PE even
                # when the vector engine is backlogged (masks/norm work)
                if c < 3:
                    cp = nc.scalar.copy
                else:
                    cp = nc.vector.tensor_copy
                if kind == "q":
                    cp(qTc[g][c][:], pq[:])
                elif kind == "k":
                    cp(kTc[g][c][:], pq[:])
                else:
                    r = 4 * c + g
                    dst = vg[r][:].rearrange("p (h c2) -> p h c2", c2=DH + 1)[:, :, 0:DH]
                    cp(dst, pq[:].rearrange("p (h c2) -> p h c2", c2=DH))

            def make_proj_thunks(c):
                # one thunk = one full psum group (atomic: alloc+MMs+evac), so
                # no other psM allocation can interleave into a live group
                th = []

                def group(kind, g):
                    pq = psM.tile([128, 512], F32, name="misc", tag="misc")
                    proj_mm(kind, g, c, pq, 0, DT)
                    proj_evac(kind, g, c, pq)

                order = ([("k", 0), ("k", 1), ("q", 0), ("v", 0), ("v", 1),
                          ("v", 2), ("q", 1), ("v", 3), ("k", 2), ("q", 2),
                          ("k", 3), ("q", 3)] if c == 0 else
                         [(kind, g) for kind in ("q", "k", "v")
                          for g in range(IT)])
                for kind, g in order:
                    th.append(partial(group, kind, g))
                return th

            # ---- output projection (as thunks) ----
            def oproj_mm(c, rb, nco, pf, i0, i1):
                ot = otg[c]
                for i in range(i0, i1):
                    nc.tensor.matmul(
                        pf[:], ot[i][:, rb * 128:(rb + 1) * 128],
                        woall[:, i * DIM + nco * 512:i * DIM + (nco + 1) * 512],
                        start=(i == 0), stop=(i == IT - 1))

            def make_oproj_thunks(c):
                th = []

                def group(rb, nco):
                    pf = psM.tile([128, 512], F32, name="misc", tag="misc")
                    oproj_mm(c, rb, nco, pf, 0, IT)
                    so = stp.tile([128, 512], BF16, name="so", tag="so")
                    nc.vector.tensor_copy(so[:], pf[:])
                    nc.sync.dma_start(
                        out[c * 512 + rb * 128:c * 512 + (rb + 1) * 128,
                            nco * 512:(nco + 1) * 512], so[:])

                for rb in range(4):
                    for nco in range(2):
                        th.append(partial(group, rb, nco))
                return th

            # ---- attention chunk (triangular skip + delayed PV: window w's
            # PV runs during window w+1, so it never waits on the exp) ----
            def emit_attention(ch, filler, per_window):
                ej = 4 * (ch + 1)
                windows = IT * ej
                rate = (len(filler.q) + 4.0) / windows
                acc = 0.0
                pend = []

                def mk_pv(pr, jb, o, pt, po_pair, first, last):
                    def pv():
                        if first:
                            po_pair.append(psPo.tile([65, 512], F32, name="po", tag="po"))
                            po_pair.append(psPo.tile([65, 512], F32, name="po", tag="po"))
                        for e in range(2):
                            h = 2 * pr + e
                            nc.tensor.matmul(
                                po_pair[e][0:65, o:512],
                                vg[jb][:, h * (DH + 1):(h + 1) * (DH + 1)],
                                pt[:, e * 512 + o:(e + 1) * 512],
                                start=first, stop=last)
                        if last:
                            # evacuate po to SBUF right away so the next
                            # pair's PV isn't blocked on psPo slots
                            raw = rawp.tile([65, 1024], PV_DT, name="raw", tag="raw")
                            nc.vector.tensor_copy(raw[:, 0:512], po_pair[0][:])
                            nc.vector.tensor_copy(raw[:, 512:1024], po_pair[1][:])
                            filler.add([partial(norm_thunk, ch, pr, raw)])
                    return pv

                for pr in range(IT):
                    po_pair = []
                    for jb in range(ej):
                        diag = jb >= ej - 4
                        o = 128 * (jb - (ej - 4)) if diag else 0
                        sp = psS.tile([128, 1024], F32, name="sp", tag="sp")
                        for e in range(2):
                            pb = 64 * e
                            nc.tensor.matmul(
                                sp[:, e * 512 + o:(e + 1) * 512],
                                kTc[pr][jb // 4][pb:pb + 64,
                                                 (jb % 4) * 128:(jb % 4 + 1) * 128],
                                qTc[pr][ch][pb:pb + 64, o:512])
                        pt = ptp.tile([128, 1024], PV_DT, name="pt", tag="pt")
                        if o:
                            spv = sp[:].rearrange("p (e q) -> p e q", q=512)[:, :, o:512]
                            ptv = pt[:].rearrange("p (e q) -> p e q", q=512)[:, :, o:512]
                            nc.scalar.activation(ptv, spv, EXPF, scale=LN2)
                        else:
                            nc.scalar.activation(pt[:], sp[:], EXPF, scale=LN2)
                        if diag:
                            # mask only the 128-wide diagonal sub-block of each half
                            for e in range(2):
                                s = e * 512 + o
                                nc.vector.tensor_mul(pt[:, s:s + 128], pt[:, s:s + 128],
                                                     mask[:, 0:128])
                        if pend:
                            pend.pop()()
                        pend.append(mk_pv(pr, jb, o, pt, po_pair,
                                          jb == 0, jb == ej - 1))
                        acc += rate
                        n = int(acc)
                        acc -= n
                        filler.drain(n)
                # flush the trailing PV before the chunk boundary so the norm
                # thunk is enqueued ahead of the next chunk's oproj fillers
                if pend:
                    pend.pop()()

            def norm_thunk(ch, pr, raw):
                # raw [65, 1024] bf16: cols 0:512 even head (dh rows 0:64, den
                # row 64), cols 512:1024 odd head. Computes NEGATED normalized
                # output (sign folded into the host-side gather).
                I16 = mybir.dt.int16
                d = raw[64:65, :]
                y0 = stp.tile([65, 1024], PV_DT, name="y0", tag="y0")
                nc.vector.tensor_scalar(
                    y0[64:65, :].bitcast(I16), d.bitcast(I16),
                    -1, 0x7EF3, op0=AL.mult, op1=AL.add)
                m = stp.tile([65, 1024], PV_DT, name="m2", tag="m2")
                nc.vector.tensor_mul(m[64:65, :], d, y0[64:65, :])
                rec = rcp.tile([65, 1024], PV_DT, name="rec", tag="rec")
                # rec = (d*y0 - 2) * y0 = -1/d (one Newton step, negated)
                nc.vector.scalar_tensor_tensor(
                    rec[64:65, :], m[64:65, :], 2.0, y0[64:65, :],
                    op0=AL.subtract, op1=AL.mult)
                ot = otg[ch][pr]
                if ch < NCH - 1:
                    # broadcast -1/den to 64 partitions on the idle gpsimd
                    # engine; keeps the PE and psM slots free. Latency is
                    # fine: these ot tiles aren't read for many microseconds.
                    # partition_broadcast sources literal partition 0, so
                    # first hop rec from partition 64 to 0 via a tiny DMA.
                    rc0 = rcp.tile([1, 1024], PV_DT, name="rc0", tag="rc0")
                    nc.sync.dma_start(rc0[0:1, :], rec[64:65, :])
                    prep = onp.tile([64, 1024], PV_DT, name="prep", tag="prep")
                    nc.gpsimd.partition_broadcast(prep[:], rc0[0:1, :])
                    on_o = onp.tile([64, 512], BF16, name="onorm", tag="onorm")
                    nc.vector.tensor_mul(on_o[:], raw[0:64, 512:1024],
                                         prep[:, 512:1024])
                    nc.sync.dma_start(ot[64:128, :], on_o[:])
                    nc.vector.tensor_mul(ot[0:64, :], raw[0:64, 0:512],
                                         prep[:, 0:512])
                else:
                    # last chunk: PE broadcast (lowest latency on the tail)
                    prep_e = psM.tile([128, 512], F32, name="misc", tag="misc")
                    nc.tensor.matmul(prep_e[0:64, :], ones1[64:65, 0:64],
                                     rec[64:65, 0:512])
                    prep_o = psM.tile([128, 512], F32, name="misc", tag="misc")
                    nc.tensor.matmul(prep_o[0:64, :], ones1[64:65, 0:64],
                                     rec[64:65, 512:1024])
                    # odd half first: its SBUF->SBUF DMA is the longer pole
                    on_o = onp.tile([64, 512], BF16, name="onorm", tag="onorm")
                    nc.vector.tensor_mul(on_o[:], raw[0:64, 512:1024],
                                         prep_o[0:64, :])
                    nc.sync.dma_start(ot[64:128, :], on_o[:])
                    nc.vector.tensor_mul(ot[0:64, :], raw[0:64, 0:512],
                                         prep_e[0:64, :])

            # ---- HAM warm-up: keep PE busy during the input-DMA wait so the
            # projection phase starts at full clock (reads uninitialized SBUF,
            # results discarded)
            # preload the exp table-set during the input-DMA wait so the first
            # real ACTIVATE doesn't pay the ~2.7us ACT_TABLE_LOAD
            dumA = big.tile([1, 64], PV_DT, name="dumA", tag="dumA")
            nc.scalar.activation(dumA[:], wupS[:], EXPF, scale=LN2)
            wupP = psM.tile([128, 512], F32, name="misc", tag="misc")
            for _ in range(150):
                nc.tensor.matmul(wupP[0:64, 0:64], wupS[0:1, :], wupS[0:1, :])

            # ---- the pipelined schedule ----
            filler = Filler()
            # chunk 0: emit only the groups attention(0, pr=0) needs up
            # front; the rest drain as filler inside the first windows
            filler.add(make_proj_thunks(0))
            filler.drain(3)

            per_window = {0: 2, 1: 1, 2: 2, 3: 2}
            oproj_sched = {3: [0, 1, 2]}
            for ch in range(NCH):
                if ch + 1 < NCH:
                    filler.add(make_proj_thunks(ch + 1))
                for c in oproj_sched.get(ch, []):
                    filler.add(make_oproj_thunks(c))
                emit_attention(ch, filler, per_window[ch])
                filler.drain_all()

            filler.add(make_oproj_thunks(NCH - 1))
            filler.drain_all()

    nc.compile()
    return nc


def kernel(x, w_qkv, w_out, b_out):
    if "nc" not in _CACHE:
        _CACHE["nc"] = _build()
    nc = _CACHE["nc"]

    x = np.asarray(x, np.float32)
    w_qkv = np.asarray(w_qkv, np.float32)
    w_out = np.asarray(w_out, np.float32)
    b_out = np.asarray(b_out, np.float32)

    # lower-triangular-inclusive mask for the diagonal 128x128 sub-block:
    # valid where q >= j
    mask = (np.arange(128)[:, None] <= np.arange(128)[None, :]).astype(np.float32)
    in_maps = []
    for c in range(8):
        b, g = c // 2, c % 2
        sl = slice(g * GI, (g + 1) * GI)
        in_maps.append(dict(
            xT=np.ascontiguousarray(x[b].T).astype(ml_dtypes.bfloat16),
            wq=(np.ascontiguousarray(w_qkv[:, sl]) * np.float32(SCALE * LOG2E)).astype(ml_dtypes.bfloat16),
            wk=np.ascontiguousarray(w_qkv[:, 1024 + g * GI:1024 + (g + 1) * GI]).astype(ml_dtypes.bfloat16),
            wv=np.ascontiguousarray(w_qkv[:, 2048 + g * GI:2048 + (g + 1) * GI]).astype(ml_dtypes.bfloat16),
            wo=np.ascontiguousarray(w_out[sl, :]).astype(ml_dtypes.bfloat16),
            msk=mask.astype(ml_dtypes.bfloat16),
            onesf=np.ones((128, 64), ml_dtypes.bfloat16),
        ))
    res = None
    for attempt in range(3):
        try:
            res = run_bass_kernel_spmd(nc, in_maps, core_ids=list(range(8)))
            break
        except Exception:
            if attempt == 2:
                raise
            time.sleep(10)
    _CACHE["res"] = res
    outs = [np.asarray(res.results[c]["out"], np.float32) for c in range(8)]
    full = np.empty((B, N, DIM), np.float32)
    for b in range(B):
        # device outputs are negated (normalization uses -1/den)
        full[b] = b_out[None, :] - outs[2 * b] - outs[2 * b + 1]
    return full


# revision 29
# speedup vs baseline: 1.1322x; 1.1322x over previous
import sys, time
from collections import deque
from functools import partial

sys.path.insert(0, "/opt/trn_rl_repo")
import numpy as np
import ml_dtypes
from concourse import bass, bacc, tile, mybir
from concourse.bass_utils import run_bass_kernel_spmd

F32 = mybir.dt.float32
BF16 = mybir.dt.bfloat16
I32 = mybir.dt.int32

B, N, DIM = 4, 2048, 1024
HEADS, DH = 16, 64
G = 8          # heads per core
GI = G * DH    # 512 = inner width per core
SCALE = DH ** -0.5
LOG2E = 1.4426950408889634
LN2 = 0.6931471805599453
NB = N // 128   # 16 j-blocks
NCH = N // 512  # 4 q-chunks
DT = DIM // 128  # 8 dim tiles
IT = GI // 128   # 4 inner tiles (= head pairs)

PV_DT = BF16

_CACHE = {}


def _build():
    nc = bacc.Bacc(None, target_bir_lowering=False)
    xT = nc.declare_dram_parameter("xT", [DIM, N], BF16, isOutput=False)
    wq = nc.declare_dram_parameter("wq", [DIM, GI], BF16, isOutput=False)
    wk = nc.declare_dram_parameter("wk", [DIM, GI], BF16, isOutput=False)
    wv = nc.declare_dram_parameter("wv", [DIM, GI], BF16, isOutput=False)
    wo = nc.declare_dram_parameter("wo", [GI, DIM], BF16, isOutput=False)
    msk = nc.declare_dram_parameter("msk", [128, 128], PV_DT, isOutput=False)
    onesf = nc.declare_dram_parameter("onesf", [128, 64], BF16, isOutput=False)
    out = nc.declare_dram_parameter("out", [N, DIM], BF16, isOutput=True)

    EXPF = mybir.ActivationFunctionType.Exp
    AL = mybir.AluOpType

    with tile.TileContext(nc) as tc:
        with (
            nc.allow_low_precision(reason="attention P/V in bf16; rel-err gate 2e-2"),
            tc.tile_pool(name="big", bufs=1) as big,
            tc.tile_pool(name="pt", bufs=6) as ptp,
            tc.tile_pool(name="st", bufs=3) as stp,
            tc.tile_pool(name="rc", bufs=3) as rcp,
            tc.tile_pool(name="raw", bufs=4) as rawp,
            tc.tile_pool(name="on", bufs=4) as onp,
            tc.tile_pool(name="psS", bufs=2, space="PSUM") as psS,
            tc.tile_pool(name="psPo", bufs=2, space="PSUM") as psPo,
            tc.tile_pool(name="psM", bufs=2, space="PSUM") as psM,
        ):
            # ---- persistent SBUF ----
            xTall = big.tile([128, DT * N], BF16, name="xTall", tag="xTall")
            wqall = big.tile([128, DT * GI], BF16, name="wqall", tag="wqall")
            wkall = big.tile([128, DT * GI], BF16, name="wkall", tag="wkall")
            wvall = big.tile([128, DT * GI], BF16, name="wvall", tag="wvall")
            woall = big.tile([128, IT * DIM], BF16, name="woall", tag="woall")
            # per-chunk q/k tiles: [ti][ch] -> [128, 512]
            qTc = [[big.tile([128, 512], BF16, name=f"q{i}_{c}", tag=f"q{i}_{c}") for c in range(NCH)]
                   for i in range(IT)]
            kTc = [[big.tile([128, 512], BF16, name=f"k{i}_{c}", tag=f"k{i}_{c}") for c in range(NCH)]
                   for i in range(IT)]
            # v with ones col per head: [128, 8*65] per j-block
            vg = [big.tile([128, G * (DH + 1)], PV_DT, name=f"v{r}", tag=f"v{r}") for r in range(NB)]
            mask = big.tile([128, 128], PV_DT, name="mask", tag="mask")
            ones1 = big.tile([128, 64], BF16, name="ones1", tag="ones1")
            # ot tiles: one generation per chunk (no cross-chunk hazards)
            otg = [[big.tile([128, 512], BF16, name=f"ot{c2}_{i}", tag=f"ot{c2}_{i}") for i in range(IT)]
                   for c2 in range(NCH)]

            # ---- input DMAs: few big strided transfers (issue cost ~600ns
            # each on the initiating engine, so fewer is much better) ----
            def _x_dst(c):
                return xTall[:].rearrange("p (d n) -> p d n", n=N)[:, :, c * 512:(c + 1) * 512]

            def _x_src(c):
                return xT.rearrange("(d p) n -> p d n", p=128)[:, :, c * 512:(c + 1) * 512]

            # x chunk 0 split across the two HW DMA queues so the first
            # k-projection can start as early as possible
            def _x_dst_h(c, h):
                return xTall[:].rearrange("p (d n) -> p d n", n=N)[
                    :, h * 4:(h + 1) * 4, c * 512:(c + 1) * 512]

            def _x_src_h(c, h):
                return xT.rearrange("(d p) n -> p d n", p=128)[
                    :, h * 4:(h + 1) * 4, c * 512:(c + 1) * 512]

            # weights split in half across both HW queues, ordered by first
            # use: wk (k0 proj) -> wq -> wv -> mask; late x chunks go via the
            # gpsimd (SWDGE) queue since they aren't needed until later
            def _w_half(dst, src, h):
                return (dst[:].rearrange("p (d c) -> p d c", c=GI)[:, h * 4:(h + 1) * 4, :],
                        src.rearrange("(d p) c -> p d c", p=128)[:, h * 4:(h + 1) * 4, :])

            nc.sync.dma_start(_x_dst_h(0, 0), _x_src_h(0, 0))
            nc.scalar.dma_start(_x_dst_h(0, 1), _x_src_h(0, 1))
            nc.sync.dma_start(*_w_half(wkall, wk, 0))
            nc.scalar.dma_start(*_w_half(wkall, wk, 1))
            nc.sync.dma_start(*_w_half(wqall, wq, 0))
            nc.scalar.dma_start(*_w_half(wqall, wq, 1))
            nc.sync.dma_start(*_w_half(wvall, wv, 0))
            nc.scalar.dma_start(*_w_half(wvall, wv, 1))
            nc.scalar.dma_start(mask[:], msk[:])
            nc.scalar.dma_start(ones1[:], onesf[:])
            wupS = big.tile([1, 64], BF16, name="wup", tag="wup")
            nc.gpsimd.memset(wupS[:], 1.0)
            for r in range(NB):
                dst = vg[r][:].rearrange("p (h c) -> p h c", c=DH + 1)[:, :, DH:DH + 1]
                nc.gpsimd.memset(dst, 1.0)
            nc.sync.dma_start(_x_dst(1), _x_src(1))
            nc.scalar.dma_start(woall[:].rearrange("p (i c) -> p i c", c=DIM),
                                wo.rearrange("(i p) c -> p i c", p=128))
            nc.gpsimd.dma_start(_x_dst(2), _x_src(2))
            nc.gpsimd.dma_start(_x_dst(3), _x_src(3))

            # ---- filler machinery ----
            class Filler:
                def __init__(self):
                    self.q = deque()

                def add(self, thunks):
                    self.q.extend(thunks)

                def drain(self, n):
                    for _ in range(n):
                        if not self.q:
                            return
                        self.q.popleft()()

                def drain_all(self):
                    while self.q:
                        self.q.popleft()()

            # ---- projection emission (as thunks) ----
            def proj_mm(kind, g, c, pq, d0, d1):
                for d in range(d0, d1):
                    if kind == "q":
                        nc.tensor.matmul(
                            pq[:],
                            wqall[:, d * GI + g * 128:d * GI + (g + 1) * 128],
                            xTall[:, d * N + c * 512:d * N + (c + 1) * 512],
                            start=(d == 0), stop=(d == DT - 1))
                    elif kind == "k":
                        nc.tensor.matmul(
                            pq[:],
                            wkall[:, d * GI + g * 128:d * GI + (g + 1) * 128],
                            xTall[:, d * N + c * 512:d * N + (c + 1) * 512],
                            start=(d == 0), stop=(d == DT - 1))
                    else:  # v: rows block r = 4c+g
                        r = 4 * c + g
                        nc.tensor.matmul(
                            pq[:],
                            xTall[:, d * N + r * 128:d * N + (r + 1) * 128],
                            wvall[:, d * GI:(d + 1) * GI],
                            start=(d == 0), stop=(d == DT - 1))

            def proj_evac(kind, g, c, pq):
                # chunks 0-2 drain while the scalar engine is mostly idle:
                # evacuating there unblocks psM slot recycling for the PE even
                # when the vector engine is backlogged (masks/norm work)
                if c < 3:
                    cp = nc.scalar.copy
                else:
                    cp = nc.vector.tensor_copy
                if kind == "q":
                    cp(qTc[g][c][:], pq[:])
                elif kind == "k":
                    cp(kTc[g][c][:], pq[:])
                else:
                    r = 4 * c + g
                    dst = vg[r][:].rearrange("p (h c2) -> p h c2", c2=DH + 1)[:, :, 0:DH]
                    cp(dst, pq[:].rearrange("p (h c2) -> p h c2", c2=DH))

            def make_proj_thunks(c):
                # one thunk = one full psum group (atomic: alloc+MMs+evac), so
                # no other psM allocation can interleave into a live group
                th = []

                def group(kind, g):
                    pq = psM.tile([128, 512], F32, name="misc", tag="misc")
                    proj_mm(kind, g, c, pq, 0, DT)
                    proj_evac(kind, g, c, pq)

                order = ([("k", 0), ("k", 1), ("q", 0), ("v", 0), ("v", 1),
                          ("v", 2), ("q", 1), ("v", 3), ("k", 2), ("q", 2),
                          ("k", 3), ("q", 3)] if c == 0 else
                         [(kind, g) for kind in ("q", "k", "v")
                          for g in range(IT)])
                for kind, g in order:
                    th.append(partial(group, kind, g))
                return th

            # ---- output projection (as thunks) ----
            def oproj_mm(c, rb, nco, pf, i0, i1):
                ot = otg[c]
                for i in range(i0, i1):
                    nc.tensor.matmul(
                        pf[:], ot[i][:, rb * 128:(rb + 1) * 128],
                        woall[:, i * DIM + nco * 512:i * DIM + (nco + 1) * 512],
                        start=(i == 0), stop=(i == IT - 1))

            def make_oproj_thunks(c):
                th = []

                def group(rb, nco):
                    pf = psM.tile([128, 512], F32, name="misc", tag="misc")
                    oproj_mm(c, rb, nco, pf, 0, IT)
                    so = stp.tile([128, 512], BF16, name="so", tag="so")
                    nc.vector.tensor_copy(so[:], pf[:])
                    nc.sync.dma_start(
                        out[c * 512 + rb * 128:c * 512 + (rb + 1) * 128,
                            nco * 512:(nco + 1) * 512], so[:])

                for rb in range(4):
                    for nco in range(2):
                        th.append(partial(group, rb, nco))
                return th

            # ---- attention chunk (triangular skip + delayed PV: window w's
            # PV runs during window w+1, so it never waits on the exp) ----
            def emit_attention(ch, filler, per_window):
                ej = 4 * (ch + 1)
                windows = IT * ej
                rate = (len(filler.q) + 4.0) / windows
                acc = 0.0
                pend = []

                def mk_pv(pr, jb, o, pt, po_pair, first, last):
                    def pv():
                        if first:
                            po_pair.append(psPo.tile([65, 512], F32, name="po", tag="po"))
                            po_pair.append(psPo.tile([65, 512], F32, name="po", tag="po"))
                        for e in range(2):
                            h = 2 * pr + e
                            nc.tensor.matmul(
                                po_pair[e][0:65, o:512],
                                vg[jb][:, h * (DH + 1):(h + 1) * (DH + 1)],
                                pt[:, e * 512 + o:(e + 1) * 512],
                                start=first, stop=last)
                        if last:
                            # evacuate po to SBUF right away so the next
                            # pair's PV isn't blocked on psPo slots
                            raw = rawp.tile([65, 1024], PV_DT, name="raw", tag="raw")
                            nc.vector.tensor_copy(raw[:, 0:512], po_pair[0][:])
                            nc.vector.tensor_copy(raw[:, 512:1024], po_pair[1][:])
                            filler.add([partial(norm_thunk, ch, pr, raw)])
                    return pv

                for pr in range(IT):
                    po_pair = []
                    for jb in range(ej):
                        diag = jb >= ej - 4
                        o = 128 * (jb - (ej - 4)) if diag else 0
                        sp = psS.tile([128, 1024], F32, name="sp", tag="sp")
                        for e in range(2):
                            pb = 64 * e
                            nc.tensor.matmul(
                                sp[:, e * 512 + o:(e + 1) * 512],
                                kTc[pr][jb // 4][pb:pb + 64,
                                                 (jb % 4) * 128:(jb % 4 + 1) * 128],
                                qTc[pr][ch][pb:pb + 64, o:512])
                        pt = ptp.tile([128, 1024], PV_DT, name="pt", tag="pt")
                        if o:
                            spv = sp[:].rearrange("p (e q) -> p e q", q=512)[:, :, o:512]
                            ptv = pt[:].rearrange("p (e q) -> p e q", q=512)[:, :, o:512]
                            nc.scalar.activation(ptv, spv, EXPF, scale=LN2)
                        else:
                            nc.scalar.activation(pt[:], sp[:], EXPF, scale=LN2)
                        if diag:
                            # mask only the 128-wide diagonal sub-block of each half
                            for e in range(2):
                                s = e * 512 + o
                                nc.vector.tensor_mul(pt[:, s:s + 128], pt[:, s:s + 128],
                                                     mask[:, 0:128])
                        if pend:
                            pend.pop()()
                        pend.append(mk_pv(pr, jb, o, pt, po_pair,
                                          jb == 0, jb == ej - 1))
                        acc += rate
                        n = int(acc)
                        acc -= n
                        filler.drain(n)
                # flush the trailing PV before the chunk boundary so the norm
                # thunk is enqueued ahead of the next chunk's oproj fillers
                if pend:
                    pend.pop()()

            def norm_thunk(ch, pr, raw):
                # raw [65, 1024] bf16: cols 0:512 even head (dh rows 0:64, den
                # row 64), cols 512:1024 odd head. Computes NEGATED normalized
                # output (sign folded into the host-side gather).
                I16 = mybir.dt.int16
                d = raw[64:65, :]
                y0 = stp.tile([65, 1024], PV_DT, name="y0", tag="y0")
                nc.vector.tensor_scalar(
                    y0[64:65, :].bitcast(I16), d.bitcast(I16),
                    -1, 0x7EF3, op0=AL.mult, op1=AL.add)
                m = stp.tile([65, 1024], PV_DT, name="m2", tag="m2")
                nc.vector.tensor_mul(m[64:65, :], d, y0[64:65, :])
                rec = rcp.tile([65, 1024], PV_DT, name="rec", tag="rec")
                # rec = (d*y0 - 2) * y0 = -1/d (one Newton step, negated)
                nc.vector.scalar_tensor_tensor(
                    rec[64:65, :], m[64:65, :], 2.0, y0[64:65, :],
                    op0=AL.subtract, op1=AL.mult)
                ot = otg[ch][pr]
                if False:
                    pass
                else:
                    # last chunk: PE broadcast (lowest latency on the tail)
                    prep_e = psM.tile([128, 512], F32, name="misc", tag="misc")
                    nc.tensor.matmul(prep_e[0:64, :], ones1[64:65, 0:64],
                                     rec[64:65, 0:512])
                    prep_o = psM.tile([128, 512], F32, name="misc", tag="misc")
                    nc.tensor.matmul(prep_o[0:64, :], ones1[64:65, 0:64],
                                     rec[64:65, 512:1024])
                    # odd half first: its SBUF->SBUF DMA is the longer pole
                    on_o = onp.tile([64, 512], BF16, name="onorm", tag="onorm")
                    nc.vector.tensor_mul(on_o[:], raw[0:64, 512:1024],
                                         prep_o[0:64, :])
                    nc.sync.dma_start(ot[64:128, :], on_o[:])
                    nc.vector.tensor_mul(ot[0:64, :], raw[0:64, 0:512],
                                         prep_e[0:64, :])

            # ---- HAM warm-up: keep PE busy during the input-DMA wait so the
            # projection phase starts at full clock (reads uninitialized SBUF,
            # results discarded)
            # preload the exp table-set during the input-DMA wait so the first
            # real ACTIVATE doesn't pay the ~2.7us ACT_TABLE_LOAD
            dumA = big.tile([1, 64], PV_DT, name="dumA", tag="dumA")
            nc.scalar.activation(dumA[:], wupS[:], EXPF, scale=LN2)
            wupP = psM.tile([128, 512], F32, name="misc", tag="misc")
            for _ in range(150):
                nc.tensor.matmul(wupP[0:64, 0:64], wupS[0:1, :], wupS[0:1, :])

            # ---- the pipelined schedule ----
            filler = Filler()
            # chunk 0: emit only the groups attention(0, pr=0) needs up
            # front; the rest drain as filler inside the first windows
            filler.add(make_proj_thunks(0))
            filler.drain(3)

            per_window = {0: 2, 1: 1, 2: 2, 3: 2}
            oproj_sched = {3: [0, 1, 2]}
            for ch in range(NCH):
                if ch + 1 < NCH:
                    filler.add(make_proj_thunks(ch + 1))
                for c in oproj_sched.get(ch, []):
                    filler.add(make_oproj_thunks(c))
                emit_attention(ch, filler, per_window[ch])
                filler.drain_all()

            filler.add(make_oproj_thunks(NCH - 1))
            filler.drain_all()

    nc.compile()
    return nc


def kernel(x, w_qkv, w_out, b_out):
    if "nc" not in _CACHE:
        _CACHE["nc"] = _build()
    nc = _CACHE["nc"]

    x = np.asarray(x, np.float32)
    w_qkv = np.asarray(w_qkv, np.float32)
    w_out = np.asarray(w_out, np.float32)
    b_out = np.asarray(b_out, np.float32)

    # lower-triangular-inclusive mask for the diagonal 128x128 sub-block:
    # valid where q >= j
    mask = (np.arange(128)[:, None] <= np.arange(128)[None, :]).astype(np.float32)
    in_maps = []
    for c in range(8):
        b, g = c // 2, c % 2
        sl = slice(g * GI, (g + 1) * GI)
        in_maps.append(dict(
            xT=np.ascontiguousarray(x[b].T).astype(ml_dtypes.bfloat16),
            wq=(np.ascontiguousarray(w_qkv[:, sl]) * np.float32(SCALE * LOG2E)).astype(ml_dtypes.bfloat16),
            wk=np.ascontiguousarray(w_qkv[:, 1024 + g * GI:1024 + (g + 1) * GI]).astype(ml_dtypes.bfloat16),
            wv=np.ascontiguousarray(w_qkv[:, 2048 + g * GI:2048 + (g + 1) * GI]).astype(ml_dtypes.bfloat16),
            wo=np.ascontiguousarray(w_out[sl, :]).astype(ml_dtypes.bfloat16),
            msk=mask.astype(ml_dtypes.bfloat16),
            onesf=np.ones((128, 64), ml_dtypes.bfloat16),
        ))
    res = None
    for attempt in range(3):
        try:
            res = run_bass_kernel_spmd(nc, in_maps, core_ids=list(range(8)))
            break
        except Exception:
            if attempt == 2:
                raise
            time.sleep(10)
    _CACHE["res"] = res
    outs = [np.asarray(res.results[c]["out"], np.float32) for c in range(8)]
    full = np.empty((B, N, DIM), np.float32)
    for b in range(B):
        # device outputs are negated (normalization uses -1/den)
        full[b] = b_out[None, :] - outs[2 * b] - outs[2 * b + 1]
    return full


# revision 39
# speedup vs baseline: 1.1513x; 1.0169x over previous
import sys, time
from collections import deque
from functools import partial

sys.path.insert(0, "/opt/trn_rl_repo")
import numpy as np
import ml_dtypes
from concourse import bass, bacc, tile, mybir
from concourse.bass_utils import run_bass_kernel_spmd

F32 = mybir.dt.float32
BF16 = mybir.dt.bfloat16
I32 = mybir.dt.int32

B, N, DIM = 4, 2048, 1024
HEADS, DH = 16, 64
G = 8          # heads per core
GI = G * DH    # 512 = inner width per core
SCALE = DH ** -0.5
LOG2E = 1.4426950408889634
LN2 = 0.6931471805599453
NB = N // 128   # 16 j-blocks
NCH = N // 512  # 4 q-chunks
DT = DIM // 128  # 8 dim tiles
IT = GI // 128   # 4 inner tiles (= head pairs)

PV_DT = BF16

_CACHE = {}


def _build():
    nc = bacc.Bacc(None, target_bir_lowering=False)
    xT = nc.declare_dram_parameter("xT", [DIM, N], BF16, isOutput=False)
    wq = nc.declare_dram_parameter("wq", [DIM, GI], BF16, isOutput=False)
    wk = nc.declare_dram_parameter("wk", [DIM, GI], BF16, isOutput=False)
    wv = nc.declare_dram_parameter("wv", [DIM, GI], BF16, isOutput=False)
    wo = nc.declare_dram_parameter("wo", [GI, DIM], BF16, isOutput=False)
    msk = nc.declare_dram_parameter("msk", [128, 128], PV_DT, isOutput=False)
    onesf = nc.declare_dram_parameter("onesf", [128, 64], BF16, isOutput=False)
    out = nc.declare_dram_parameter("out", [N, DIM], BF16, isOutput=True)

    EXPF = mybir.ActivationFunctionType.Exp
    AL = mybir.AluOpType

    with tile.TileContext(nc) as tc:
        with (
            nc.allow_low_precision(reason="attention P/V in bf16; rel-err gate 2e-2"),
            tc.tile_pool(name="big", bufs=1) as big,
            tc.tile_pool(name="pt", bufs=6) as ptp,
            tc.tile_pool(name="st", bufs=3) as stp,
            tc.tile_pool(name="rc", bufs=3) as rcp,
            tc.tile_pool(name="raw", bufs=4) as rawp,
            tc.tile_pool(name="on", bufs=4) as onp,
            tc.tile_pool(name="psS", bufs=2, space="PSUM") as psS,
            tc.tile_pool(name="psPo", bufs=2, space="PSUM") as psPo,
            tc.tile_pool(name="psM", bufs=2, space="PSUM") as psM,
        ):
            # ---- persistent SBUF ----
            xTall = big.tile([128, DT * N], BF16, name="xTall", tag="xTall")
            wqall = big.tile([128, DT * GI], BF16, name="wqall", tag="wqall")
            wkall = big.tile([128, DT * GI], BF16, name="wkall", tag="wkall")
            wvall = big.tile([128, DT * GI], BF16, name="wvall", tag="wvall")
            woall = big.tile([128, IT * DIM], BF16, name="woall", tag="woall")
            # per-chunk q/k tiles: [ti][ch] -> [128, 512]
            qTc = [[big.tile([128, 512], BF16, name=f"q{i}_{c}", tag=f"q{i}_{c}") for c in range(NCH)]
                   for i in range(IT)]
            kTc = [[big.tile([128, 512], BF16, name=f"k{i}_{c}", tag=f"k{i}_{c}") for c in range(NCH)]
                   for i in range(IT)]
            # v with ones col per head: [128, 8*65] per j-block
            vg = [big.tile([128, G * (DH + 1)], PV_DT, name=f"v{r}", tag=f"v{r}") for r in range(NB)]
            mask = big.tile([128, 128], PV_DT, name="mask", tag="mask")
            ones1 = big.tile([128, 64], BF16, name="ones1", tag="ones1")
            # ot tiles: one generation per chunk (no cross-chunk hazards)
            otg = [[big.tile([128, 512], BF16, name=f"ot{c2}_{i}", tag=f"ot{c2}_{i}") for i in range(IT)]
                   for c2 in range(NCH)]

            # ---- input DMAs: few big strided transfers (issue cost ~600ns
            # each on the initiating engine, so fewer is much better) ----
            def _x_dst(c):
                return xTall[:].rearrange("p (d n) -> p d n", n=N)[:, :, c * 512:(c + 1) * 512]

            def _x_src(c):
                return xT.rearrange("(d p) n -> p d n", p=128)[:, :, c * 512:(c + 1) * 512]

            # x chunk 0 split across the two HW DMA queues so the first
            # k-projection can start as early as possible
            def _x_dst_h(c, h):
                return xTall[:].rearrange("p (d n) -> p d n", n=N)[
                    :, h * 4:(h + 1) * 4, c * 512:(c + 1) * 512]

            def _x_src_h(c, h):
                return xT.rearrange("(d p) n -> p d n", p=128)[
                    :, h * 4:(h + 1) * 4, c * 512:(c + 1) * 512]

            # weights split in half across both HW queues, ordered by first
            # use: wk (k0 proj) -> wq -> wv -> mask; late x chunks go via the
            # gpsimd (SWDGE) queue since they aren't needed until later
            def _w_half(dst, src, h):
                return (dst[:].rearrange("p (d c) -> p d c", c=GI)[:, h * 4:(h + 1) * 4, :],
                        src.rearrange("(d p) c -> p d c", p=128)[:, h * 4:(h + 1) * 4, :])

            nc.sync.dma_start(_x_dst_h(0, 0), _x_src_h(0, 0))
            nc.scalar.dma_start(_x_dst_h(0, 1), _x_src_h(0, 1))
            nc.sync.dma_start(*_w_half(wkall, wk, 0))
            nc.scalar.dma_start(*_w_half(wkall, wk, 1))
            nc.sync.dma_start(*_w_half(wqall, wq, 0))
            nc.scalar.dma_start(*_w_half(wqall, wq, 1))
            nc.sync.dma_start(*_w_half(wvall, wv, 0))
            nc.scalar.dma_start(*_w_half(wvall, wv, 1))
            nc.scalar.dma_start(mask[:], msk[:])
            nc.scalar.dma_start(ones1[:], onesf[:])
            wupS = big.tile([1, 64], BF16, name="wup", tag="wup")
            nc.gpsimd.memset(wupS[:], 1.0)
            for r in range(NB):
                dst = vg[r][:].rearrange("p (h c) -> p h c", c=DH + 1)[:, :, DH:DH + 1]
                nc.gpsimd.memset(dst, 1.0)
            nc.sync.dma_start(_x_dst(1), _x_src(1))
            nc.scalar.dma_start(woall[:].rearrange("p (i c) -> p i c", c=DIM),
                                wo.rearrange("(i p) c -> p i c", p=128))
            nc.gpsimd.dma_start(_x_dst(2), _x_src(2))
            nc.gpsimd.dma_start(_x_dst(3), _x_src(3))

            # ---- filler machinery ----
            class Filler:
                def __init__(self):
                    self.q = deque()

                def add(self, thunks):
                    self.q.extend(thunks)

                def drain(self, n):
                    for _ in range(n):
                        if not self.q:
                            return
                        self.q.popleft()()

                def drain_all(self):
                    while self.q:
                        self.q.popleft()()

            # ---- projection emission (as thunks) ----
            def proj_mm(kind, g, c, pq, d0, d1):
                for d in range(d0, d1):
                    if kind == "q":
                        nc.tensor.matmul(
                            pq[:],
                            wqall[:, d * GI + g * 128:d * GI + (g + 1) * 128],
                            xTall[:, d * N + c * 512:d * N + (c + 1) * 512],
                            start=(d == 0), stop=(d == DT - 1))
                    elif kind == "k":
                        nc.tensor.matmul(
                            pq[:],
                            wkall[:, d * GI + g * 128:d * GI + (g + 1) * 128],
                            xTall[:, d * N + c * 512:d * N + (c + 1) * 512],
                            start=(d == 0), stop=(d == DT - 1))
                    else:  # v: rows block r = 4c+g
                        r = 4 * c + g
                        nc.tensor.matmul(
                            pq[:],
                            xTall[:, d * N + r * 128:d * N + (r + 1) * 128],
                            wvall[:, d * GI:(d + 1) * GI],
                            start=(d == 0), stop=(d == DT - 1))

            def proj_evac(kind, g, c, pq):
                # proj thunks drain while the scalar engine has idle slack
                # (chunk c drains during attention c-1): evacuating there
                # unblocks psM slot recycling for the PE even when the vector
                # engine is backlogged (masks/norm work)
                cp = nc.scalar.copy
                if kind == "q":
                    cp(qTc[g][c][:], pq[:])
                elif kind == "k":
                    cp(kTc[g][c][:], pq[:])
                else:
                    r = 4 * c + g
                    dst = vg[r][:].rearrange("p (h c2) -> p h c2", c2=DH + 1)[:, :, 0:DH]
                    cp(dst, pq[:].rearrange("p (h c2) -> p h c2", c2=DH))

            def make_proj_thunks(c):
                # one thunk = one full psum group (atomic: alloc+MMs+evac), so
                # no other psM allocation can interleave into a live group
                th = []

                def group(kind, g):
                    pq = psM.tile([128, 512], F32, name="misc", tag="misc")
                    proj_mm(kind, g, c, pq, 0, DT)
                    proj_evac(kind, g, c, pq)

                # c0: k0+q0+k1 gate the first window (all wk/wq-only); v0
                # must drain in window 0, one window before the delayed PV
                # of window 0 reads it
                order = ([("k", 0), ("q", 0), ("k", 1), ("v", 0), ("v", 1),
                          ("v", 2), ("q", 1), ("v", 3), ("k", 2), ("q", 2),
                          ("k", 3), ("q", 3)] if c == 0 else
                         [(kind, g) for kind in ("q", "k", "v")
                          for g in range(IT)])
                for kind, g in order:
                    th.append(partial(group, kind, g))
                return th

            # ---- output projection (as thunks) ----
            def oproj_mm(c, rb, nco, pf, i0, i1):
                ot = otg[c]
                for i in range(i0, i1):
                    nc.tensor.matmul(
                        pf[:], ot[i][:, rb * 128:(rb + 1) * 128],
                        woall[:, i * DIM + nco * 512:i * DIM + (nco + 1) * 512],
                        start=(i == 0), stop=(i == IT - 1))

            def make_oproj_thunks(c, evac_scalar=False):
                th = []

                def group(rb, nco):
                    pf = psM.tile([128, 512], F32, name="misc", tag="misc")
                    oproj_mm(c, rb, nco, pf, 0, IT)
                    so = stp.tile([128, 512], BF16, name="so", tag="so")
                    if evac_scalar:
                        # tail: ACT is idle and DVE-gated psM recycling is
                        # the group-to-group critical path
                        nc.scalar.copy(so[:], pf[:])
                    else:
                        nc.vector.tensor_copy(so[:], pf[:])
                    nc.sync.dma_start(
                        out[c * 512 + rb * 128:c * 512 + (rb + 1) * 128,
                            nco * 512:(nco + 1) * 512], so[:])

                for rb in range(4):
                    for nco in range(2):
                        th.append(partial(group, rb, nco))
                return th

            # ---- attention chunk (triangular skip + delayed PV: window w's
            # PV runs during window w+1, so it never waits on the exp) ----
            def emit_attention(ch, filler, per_window, pend):
                ej = 4 * (ch + 1)
                windows = IT * ej
                rate = (len(filler.q) + 4.0) / windows
                acc = 0.0

                def mk_pv(pr, jb, o, pt, po_pair, first, last):
                    def pv():
                        if first:
                            po_pair.append(psPo.tile([65, 512], F32, name="po", tag="po"))
                            po_pair.append(psPo.tile([65, 512], F32, name="po", tag="po"))
                        for e in range(2):
                            h = 2 * pr + e
                            nc.tensor.matmul(
                                po_pair[e][0:65, o:512],
                                vg[jb][:, h * (DH + 1):(h + 1) * (DH + 1)],
                                pt[:, e * 512 + o:(e + 1) * 512],
                                start=first, stop=last)
                        if last:
                            # evacuate po to SBUF right away so the next
                            # pair's PV isn't blocked on psPo slots
                            raw = rawp.tile([65, 1024], PV_DT, name="raw", tag="raw")
                            nc.vector.tensor_copy(raw[:, 0:512], po_pair[0][:])
                            nc.vector.tensor_copy(raw[:, 512:1024], po_pair[1][:])
                            filler.add([partial(norm_thunk, ch, pr, raw)])
                    return pv

                for pr in range(IT):
                    po_pair = []
                    for jb in range(ej):
                        diag = jb >= ej - 4
                        o = 128 * (jb - (ej - 4)) if diag else 0
                        sp = psS.tile([128, 1024], F32, name="sp", tag="sp")
                        for e in range(2):
                            pb = 64 * e
                            nc.tensor.matmul(
                                sp[:, e * 512 + o:(e + 1) * 512],
                                kTc[pr][jb // 4][pb:pb + 64,
                                                 (jb % 4) * 128:(jb % 4 + 1) * 128],
                                qTc[pr][ch][pb:pb + 64, o:512])
                        pt = ptp.tile([128, 1024], PV_DT, name="pt", tag="pt")
                        if o:
                            spv = sp[:].rearrange("p (e q) -> p e q", q=512)[:, :, o:512]
                            ptv = pt[:].rearrange("p (e q) -> p e q", q=512)[:, :, o:512]
                            nc.scalar.activation(ptv, spv, EXPF, scale=LN2)
                        else:
                            nc.scalar.activation(pt[:], sp[:], EXPF, scale=LN2)
                        if diag:
                            # mask only the 128-wide diagonal sub-block of each half
                            for e in range(2):
                                s = e * 512 + o
                                nc.vector.tensor_mul(pt[:, s:s + 128], pt[:, s:s + 128],
                                                     mask[:, 0:128])
                        if pend:
                            pend.pop()()
                        pend.append(mk_pv(pr, jb, o, pt, po_pair,
                                          jb == 0, jb == ej - 1))
                        acc += rate
                        n = int(acc)
                        acc -= n
                        filler.drain(n)

            def norm_thunk(ch, pr, raw):
                # raw [65, 1024] bf16: cols 0:512 even head (dh rows 0:64, den
                # row 64), cols 512:1024 odd head. Computes NEGATED normalized
                # output (sign folded into the host-side gather).
                I16 = mybir.dt.int16
                d = raw[64:65, :]
                y0 = stp.tile([65, 1024], PV_DT, name="y0", tag="y0")
                nc.vector.tensor_scalar(
                    y0[64:65, :].bitcast(I16), d.bitcast(I16),
                    -1, 0x7EF3, op0=AL.mult, op1=AL.add)
                m = stp.tile([65, 1024], PV_DT, name="m2", tag="m2")
                nc.vector.tensor_mul(m[64:65, :], d, y0[64:65, :])
                rec = rcp.tile([65, 1024], PV_DT, name="rec", tag="rec")
                # rec = (d*y0 - 2) * y0 = -1/d (one Newton step, negated)
                nc.vector.scalar_tensor_tensor(
                    rec[64:65, :], m[64:65, :], 2.0, y0[64:65, :],
                    op0=AL.subtract, op1=AL.mult)
                ot = otg[ch][pr]
                if False:
                    pass
                else:
                    # last chunk: PE broadcast (lowest latency on the tail)
                    prep_e = psM.tile([128, 512], F32, name="misc", tag="misc")
                    nc.tensor.matmul(prep_e[0:64, :], ones1[64:65, 0:64],
                                     rec[64:65, 0:512])
                    prep_o = psM.tile([128, 512], F32, name="misc", tag="misc")
                    nc.tensor.matmul(prep_o[0:64, :], ones1[64:65, 0:64],
                                     rec[64:65, 512:1024])
                    # odd half first: its SBUF->SBUF DMA is the longer pole
                    on_o = onp.tile([64, 512], BF16, name="onorm", tag="onorm")
                    nc.vector.tensor_mul(on_o[:], raw[0:64, 512:1024],
                                         prep_o[0:64, :])
                    nc.sync.dma_start(ot[64:128, :], on_o[:])
                    nc.vector.tensor_mul(ot[0:64, :], raw[0:64, 0:512],
                                         prep_e[0:64, :])

            # ---- HAM warm-up: keep PE busy during the input-DMA wait so the
            # projection phase starts at full clock (reads uninitialized SBUF,
            # results discarded)
            # preload the exp table-set during the input-DMA wait so the first
            # real ACTIVATE doesn't pay the ~2.7us ACT_TABLE_LOAD
            dumA = big.tile([1, 64], PV_DT, name="dumA", tag="dumA")
            nc.scalar.activation(dumA[:], wupS[:], EXPF, scale=LN2)
            wupP = psM.tile([128, 512], F32, name="misc", tag="misc")
            for _ in range(150):
                nc.tensor.matmul(wupP[0:64, 0:64], wupS[0:1, :], wupS[0:1, :])

            # ---- the pipelined schedule ----
            filler = Filler()
            # chunk 0: emit only the groups attention(0, pr=0) needs up
            # front; the rest drain as filler inside the first windows
            filler.add(make_proj_thunks(0))
            filler.drain(3)

            per_window = {0: 2, 1: 1, 2: 2, 3: 2}
            oproj_sched = {3: [0, 1, 2]}
            # the delayed PV of each chunk's last window carries into the
            # next chunk's first window; it must be flushed only before
            # oproj fillers join the queue (they read the ot tiles the
            # carried norm writes)
            pend = []
            for ch in range(NCH):
                if ch + 1 < NCH:
                    filler.add(make_proj_thunks(ch + 1))
                ops = oproj_sched.get(ch, [])
                if ops and pend:
                    pend.pop()()
                for c in ops:
                    filler.add(make_oproj_thunks(c))
                emit_attention(ch, filler, per_window[ch], pend)
                filler.drain_all()

            if pend:
                pend.pop()()
            filler.add(make_oproj_thunks(NCH - 1, evac_scalar=True))
            filler.drain_all()

    nc.compile()
    return nc


def kernel(x, w_qkv, w_out, b_out):
    if "nc" not in _CACHE:
        _CACHE["nc"] = _build()
    nc = _CACHE["nc"]

    x = np.asarray(x, np.float32)
    w_qkv = np.asarray(w_qkv, np.float32)
    w_out = np.asarray(w_out, np.float32)
    b_out = np.asarray(b_out, np.float32)

    # lower-triangular-inclusive mask for the diagonal 128x128 sub-block:
    # valid where q >= j
    mask = (np.arange(128)[:, None] <= np.arange(128)[None, :]).astype(np.float32)
    in_maps = []
    for c in range(8):
        b, g = c // 2, c % 2
        sl = slice(g * GI, (g + 1) * GI)
        in_maps.append(dict(
            xT=np.ascontiguousarray(x[b].T).astype(ml_dtypes.bfloat16),
            wq=(np.ascontiguousarray(w_qkv[:, sl]) * np.float32(SCALE * LOG2E)).astype(ml_dtypes.bfloat16),
            wk=np.ascontiguousarray(w_qkv[:, 1024 + g * GI:1024 + (g + 1) * GI]).astype(ml_dtypes.bfloat16),
            wv=np.ascontiguousarray(w_qkv[:, 2048 + g * GI:2048 + (g + 1) * GI]).astype(ml_dtypes.bfloat16),
            wo=np.ascontiguousarray(w_out[sl, :]).astype(ml_dtypes.bfloat16),
            msk=mask.astype(ml_dtypes.bfloat16),
            onesf=np.ones((128, 64), ml_dtypes.bfloat16),
        ))
    res = None
    for attempt in range(3):
        try:
            res = run_bass_kernel_spmd(nc, in_maps, core_ids=list(range(8)))
            break
        except Exception:
            if attempt == 2:
                raise
            time.sleep(10)
    _CACHE["res"] = res
    outs = [np.asarray(res.results[c]["out"], np.float32) for c in range(8)]
    full = np.empty((B, N, DIM), np.float32)
    for b in range(B):
        # device outputs are negated (normalization uses -1/den)
        full[b] = b_out[None, :] - outs[2 * b] - outs[2 * b + 1]
    return full


# revision 42
# speedup vs baseline: 1.1578x; 1.0057x over previous
import sys, time
from collections import deque
from functools import partial

sys.path.insert(0, "/opt/trn_rl_repo")
import numpy as np
import ml_dtypes
from concourse import bass, bacc, tile, mybir
from concourse.bass_utils import run_bass_kernel_spmd

F32 = mybir.dt.float32
BF16 = mybir.dt.bfloat16
I32 = mybir.dt.int32

B, N, DIM = 4, 2048, 1024
HEADS, DH = 16, 64
G = 8          # heads per core
GI = G * DH    # 512 = inner width per core
SCALE = DH ** -0.5
LOG2E = 1.4426950408889634
LN2 = 0.6931471805599453
NB = N // 128   # 16 j-blocks
NCH = N // 512  # 4 q-chunks
DT = DIM // 128  # 8 dim tiles
IT = GI // 128   # 4 inner tiles (= head pairs)

PV_DT = BF16

_CACHE = {}


def _build():
    nc = bacc.Bacc(None, target_bir_lowering=False)
    xT = nc.declare_dram_parameter("xT", [DIM, N], BF16, isOutput=False)
    wq = nc.declare_dram_parameter("wq", [DIM, GI], BF16, isOutput=False)
    wk = nc.declare_dram_parameter("wk", [DIM, GI], BF16, isOutput=False)
    wv = nc.declare_dram_parameter("wv", [DIM, GI], BF16, isOutput=False)
    wo = nc.declare_dram_parameter("wo", [GI, DIM], BF16, isOutput=False)
    msk = nc.declare_dram_parameter("msk", [128, 128], PV_DT, isOutput=False)
    onesf = nc.declare_dram_parameter("onesf", [128, 64], BF16, isOutput=False)
    out = nc.declare_dram_parameter("out", [N, DIM], BF16, isOutput=True)

    EXPF = mybir.ActivationFunctionType.Exp
    AL = mybir.AluOpType

    with tile.TileContext(nc) as tc:
        with (
            nc.allow_low_precision(reason="attention P/V in bf16; rel-err gate 2e-2"),
            tc.tile_pool(name="big", bufs=1) as big,
            tc.tile_pool(name="pt", bufs=6) as ptp,
            tc.tile_pool(name="st", bufs=3) as stp,
            tc.tile_pool(name="rc", bufs=3) as rcp,
            tc.tile_pool(name="raw", bufs=4) as rawp,
            tc.tile_pool(name="on", bufs=4) as onp,
            tc.tile_pool(name="psS", bufs=2, space="PSUM") as psS,
            tc.tile_pool(name="psPo", bufs=2, space="PSUM") as psPo,
            tc.tile_pool(name="psM", bufs=2, space="PSUM") as psM,
        ):
            # ---- persistent SBUF ----
            xTall = big.tile([128, DT * N], BF16, name="xTall", tag="xTall")
            wqall = big.tile([128, DT * GI], BF16, name="wqall", tag="wqall")
            wkall = big.tile([128, DT * GI], BF16, name="wkall", tag="wkall")
            wvall = big.tile([128, DT * GI], BF16, name="wvall", tag="wvall")
            woall = big.tile([128, IT * DIM], BF16, name="woall", tag="woall")
            # per-chunk q/k tiles: [ti][ch] -> [128, 512]
            qTc = [[big.tile([128, 512], BF16, name=f"q{i}_{c}", tag=f"q{i}_{c}") for c in range(NCH)]
                   for i in range(IT)]
            kTc = [[big.tile([128, 512], BF16, name=f"k{i}_{c}", tag=f"k{i}_{c}") for c in range(NCH)]
                   for i in range(IT)]
            # v with ones col per head: [128, 8*65] per j-block
            vg = [big.tile([128, G * (DH + 1)], PV_DT, name=f"v{r}", tag=f"v{r}") for r in range(NB)]
            mask = big.tile([128, 128], PV_DT, name="mask", tag="mask")
            ones1 = big.tile([128, 64], BF16, name="ones1", tag="ones1")
            # ot tiles: one generation per chunk (no cross-chunk hazards)
            otg = [[big.tile([128, 512], BF16, name=f"ot{c2}_{i}", tag=f"ot{c2}_{i}") for i in range(IT)]
                   for c2 in range(NCH)]

            # ---- input DMAs: few big strided transfers (issue cost ~600ns
            # each on the initiating engine, so fewer is much better) ----
            def _x_dst(c):
                return xTall[:].rearrange("p (d n) -> p d n", n=N)[:, :, c * 512:(c + 1) * 512]

            def _x_src(c):
                return xT.rearrange("(d p) n -> p d n", p=128)[:, :, c * 512:(c + 1) * 512]

            # x chunk 0 split across the two HW DMA queues so the first
            # k-projection can start as early as possible
            def _x_dst_h(c, h):
                return xTall[:].rearrange("p (d n) -> p d n", n=N)[
                    :, h * 4:(h + 1) * 4, c * 512:(c + 1) * 512]

            def _x_src_h(c, h):
                return xT.rearrange("(d p) n -> p d n", p=128)[
                    :, h * 4:(h + 1) * 4, c * 512:(c + 1) * 512]

            # weights split in half across both HW queues, ordered by first
            # use: wk (k0 proj) -> wq -> wv -> mask; late x chunks go via the
            # gpsimd (SWDGE) queue since they aren't needed until later
            def _w_half(dst, src, h):
                return (dst[:].rearrange("p (d c) -> p d c", c=GI)[:, h * 4:(h + 1) * 4, :],
                        src.rearrange("(d p) c -> p d c", p=128)[:, h * 4:(h + 1) * 4, :])

            nc.sync.dma_start(_x_dst_h(0, 0), _x_src_h(0, 0))
            nc.scalar.dma_start(_x_dst_h(0, 1), _x_src_h(0, 1))
            nc.sync.dma_start(*_w_half(wkall, wk, 0))
            nc.scalar.dma_start(*_w_half(wkall, wk, 1))
            nc.sync.dma_start(*_w_half(wqall, wq, 0))
            nc.scalar.dma_start(*_w_half(wqall, wq, 1))
            nc.sync.dma_start(*_w_half(wvall, wv, 0))
            nc.scalar.dma_start(*_w_half(wvall, wv, 1))
            nc.scalar.dma_start(mask[:], msk[:])
            nc.scalar.dma_start(ones1[:], onesf[:])
            wupS = big.tile([1, 64], BF16, name="wup", tag="wup")
            nc.gpsimd.memset(wupS[:], 1.0)
            for r in range(NB):
                dst = vg[r][:].rearrange("p (h c) -> p h c", c=DH + 1)[:, :, DH:DH + 1]
                nc.gpsimd.memset(dst, 1.0)
            nc.sync.dma_start(_x_dst(1), _x_src(1))
            nc.scalar.dma_start(woall[:].rearrange("p (i c) -> p i c", c=DIM),
                                wo.rearrange("(i p) c -> p i c", p=128))
            nc.gpsimd.dma_start(_x_dst(2), _x_src(2))
            nc.gpsimd.dma_start(_x_dst(3), _x_src(3))

            # ---- filler machinery ----
            class Filler:
                def __init__(self):
                    self.q = deque()

                def add(self, thunks):
                    self.q.extend(thunks)

                def drain(self, n):
                    for _ in range(n):
                        if not self.q:
                            return
                        self.q.popleft()()

                def drain_all(self):
                    while self.q:
                        self.q.popleft()()

            # ---- projection emission (as thunks) ----
            def proj_mm(kind, g, c, pq, d0, d1):
                for d in range(d0, d1):
                    if kind == "q":
                        nc.tensor.matmul(
                            pq[:],
                            wqall[:, d * GI + g * 128:d * GI + (g + 1) * 128],
                            xTall[:, d * N + c * 512:d * N + (c + 1) * 512],
                            start=(d == 0), stop=(d == DT - 1))
                    elif kind == "k":
                        nc.tensor.matmul(
                            pq[:],
                            wkall[:, d * GI + g * 128:d * GI + (g + 1) * 128],
                            xTall[:, d * N + c * 512:d * N + (c + 1) * 512],
                            start=(d == 0), stop=(d == DT - 1))
                    else:  # v: rows block r = 4c+g
                        r = 4 * c + g
                        nc.tensor.matmul(
                            pq[:],
                            xTall[:, d * N + r * 128:d * N + (r + 1) * 128],
                            wvall[:, d * GI:(d + 1) * GI],
                            start=(d == 0), stop=(d == DT - 1))

            def proj_evac(kind, g, c, pq):
                # proj thunks drain while the scalar engine has idle slack
                # (chunk c drains during attention c-1): evacuating there
                # unblocks psM slot recycling for the PE even when the vector
                # engine is backlogged (masks/norm work)
                cp = nc.scalar.copy
                if kind == "q":
                    cp(qTc[g][c][:], pq[:])
                elif kind == "k":
                    cp(kTc[g][c][:], pq[:])
                else:
                    r = 4 * c + g
                    dst = vg[r][:].rearrange("p (h c2) -> p h c2", c2=DH + 1)[:, :, 0:DH]
                    cp(dst, pq[:].rearrange("p (h c2) -> p h c2", c2=DH))

            def make_proj_thunks(c):
                # one thunk = one full psum group (atomic: alloc+MMs+evac), so
                # no other psM allocation can interleave into a live group
                th = []

                def group(kind, g):
                    pq = psM.tile([128, 512], F32, name="misc", tag="misc")
                    proj_mm(kind, g, c, pq, 0, DT)
                    proj_evac(kind, g, c, pq)

                # c0: all k groups run first (they need only wk, which lands
                # before wq) so the PE stays busy while wq is in flight; v0
                # must drain in window 0, one window before the delayed PV
                # of window 0 reads it
                order = ([("k", 0), ("k", 1), ("k", 2), ("k", 3), ("q", 0),
                          ("v", 0), ("v", 1), ("v", 2), ("q", 1), ("v", 3),
                          ("q", 2), ("q", 3)] if c == 0 else
                         [(kind, g) for kind in ("q", "k", "v")
                          for g in range(IT)])
                for kind, g in order:
                    th.append(partial(group, kind, g))
                return th

            # ---- output projection (as thunks) ----
            def oproj_mm(c, rb, nco, pf, i0, i1):
                ot = otg[c]
                for i in range(i0, i1):
                    nc.tensor.matmul(
                        pf[:], ot[i][:, rb * 128:(rb + 1) * 128],
                        woall[:, i * DIM + nco * 512:i * DIM + (nco + 1) * 512],
                        start=(i == 0), stop=(i == IT - 1))

            def make_oproj_thunks(c, evac_scalar=False):
                th = []

                def group(rb, nco):
                    pf = psM.tile([128, 512], F32, name="misc", tag="misc")
                    oproj_mm(c, rb, nco, pf, 0, IT)
                    so = stp.tile([128, 512], BF16, name="so", tag="so")
                    if evac_scalar:
                        # tail: ACT is idle and DVE-gated psM recycling is
                        # the group-to-group critical path
                        nc.scalar.copy(so[:], pf[:])
                    else:
                        nc.vector.tensor_copy(so[:], pf[:])
                    nc.sync.dma_start(
                        out[c * 512 + rb * 128:c * 512 + (rb + 1) * 128,
                            nco * 512:(nco + 1) * 512], so[:])

                for rb in range(4):
                    for nco in range(2):
                        th.append(partial(group, rb, nco))
                return th

            # ---- attention chunk (triangular skip + delayed PV: window w's
            # PV runs during window w+1, so it never waits on the exp) ----
            def emit_attention(ch, filler, per_window, pend):
                ej = 4 * (ch + 1)
                windows = IT * ej
                rate = (len(filler.q) + 4.0) / windows
                acc = 0.0

                def mk_pv(pr, jb, o, pt, po_pair, first, last):
                    def pv():
                        if first:
                            po_pair.append(psPo.tile([65, 512], F32, name="po", tag="po"))
                            po_pair.append(psPo.tile([65, 512], F32, name="po", tag="po"))
                        for e in range(2):
                            h = 2 * pr + e
                            nc.tensor.matmul(
                                po_pair[e][0:65, o:512],
                                vg[jb][:, h * (DH + 1):(h + 1) * (DH + 1)],
                                pt[:, e * 512 + o:(e + 1) * 512],
                                start=first, stop=last)
                        if last:
                            # evacuate po to SBUF right away so the next
                            # pair's PV isn't blocked on psPo slots
                            raw = rawp.tile([65, 1024], PV_DT, name="raw", tag="raw")
                            if ch == NCH - 1 and pr == IT - 1:
                                # very last pair: this chain is the kernel
                                # tail; split the two evacs across scalar and
                                # vector (parallel PSUM reads, ACT is idle)
                                nc.scalar.copy(raw[:, 0:512], po_pair[0][:])
                            else:
                                nc.vector.tensor_copy(raw[:, 0:512], po_pair[0][:])
                            nc.vector.tensor_copy(raw[:, 512:1024], po_pair[1][:])
                            filler.add([partial(norm_thunk, ch, pr, raw)])
                    return pv

                for pr in range(IT):
                    po_pair = []
                    for jb in range(ej):
                        diag = jb >= ej - 4
                        o = 128 * (jb - (ej - 4)) if diag else 0
                        sp = psS.tile([128, 1024], F32, name="sp", tag="sp")
                        for e in range(2):
                            pb = 64 * e
                            nc.tensor.matmul(
                                sp[:, e * 512 + o:(e + 1) * 512],
                                kTc[pr][jb // 4][pb:pb + 64,
                                                 (jb % 4) * 128:(jb % 4 + 1) * 128],
                                qTc[pr][ch][pb:pb + 64, o:512])
                        pt = ptp.tile([128, 1024], PV_DT, name="pt", tag="pt")
                        if o:
                            spv = sp[:].rearrange("p (e q) -> p e q", q=512)[:, :, o:512]
                            ptv = pt[:].rearrange("p (e q) -> p e q", q=512)[:, :, o:512]
                            nc.scalar.activation(ptv, spv, EXPF, scale=LN2)
                        else:
                            nc.scalar.activation(pt[:], sp[:], EXPF, scale=LN2)
                        if diag:
                            # mask only the 128-wide diagonal sub-block of each half
                            for e in range(2):
                                s = e * 512 + o
                                nc.vector.tensor_mul(pt[:, s:s + 128], pt[:, s:s + 128],
                                                     mask[:, 0:128])
                        if pend:
                            pend.pop()()
                        pend.append(mk_pv(pr, jb, o, pt, po_pair,
                                          jb == 0, jb == ej - 1))
                        acc += rate
                        n = int(acc)
                        acc -= n
                        filler.drain(n)

            def norm_thunk(ch, pr, raw):
                # raw [65, 1024] bf16: cols 0:512 even head (dh rows 0:64, den
                # row 64), cols 512:1024 odd head. Computes NEGATED normalized
                # output (sign folded into the host-side gather).
                I16 = mybir.dt.int16
                d = raw[64:65, :]
                y0 = stp.tile([65, 1024], PV_DT, name="y0", tag="y0")
                nc.vector.tensor_scalar(
                    y0[64:65, :].bitcast(I16), d.bitcast(I16),
                    -1, 0x7EF3, op0=AL.mult, op1=AL.add)
                m = stp.tile([65, 1024], PV_DT, name="m2", tag="m2")
                nc.vector.tensor_mul(m[64:65, :], d, y0[64:65, :])
                rec = rcp.tile([65, 1024], PV_DT, name="rec", tag="rec")
                # rec = (d*y0 - 2) * y0 = -1/d (one Newton step, negated)
                nc.vector.scalar_tensor_tensor(
                    rec[64:65, :], m[64:65, :], 2.0, y0[64:65, :],
                    op0=AL.subtract, op1=AL.mult)
                ot = otg[ch][pr]
                if False:
                    pass
                else:
                    # last chunk: PE broadcast (lowest latency on the tail)
                    prep_e = psM.tile([128, 512], F32, name="misc", tag="misc")
                    nc.tensor.matmul(prep_e[0:64, :], ones1[64:65, 0:64],
                                     rec[64:65, 0:512])
                    prep_o = psM.tile([128, 512], F32, name="misc", tag="misc")
                    nc.tensor.matmul(prep_o[0:64, :], ones1[64:65, 0:64],
                                     rec[64:65, 512:1024])
                    # odd half first: its SBUF->SBUF DMA is the longer pole
                    on_o = onp.tile([64, 512], BF16, name="onorm", tag="onorm")
                    nc.vector.tensor_mul(on_o[:], raw[0:64, 512:1024],
                                         prep_o[0:64, :])
                    nc.sync.dma_start(ot[64:128, :], on_o[:])
                    nc.vector.tensor_mul(ot[0:64, :], raw[0:64, 0:512],
                                         prep_e[0:64, :])

            # ---- HAM warm-up: keep PE busy during the input-DMA wait so the
            # projection phase starts at full clock (reads uninitialized SBUF,
            # results discarded)
            # preload the exp table-set during the input-DMA wait so the first
            # real ACTIVATE doesn't pay the ~2.7us ACT_TABLE_LOAD
            dumA = big.tile([1, 64], PV_DT, name="dumA", tag="dumA")
            nc.scalar.activation(dumA[:], wupS[:], EXPF, scale=LN2)
            wupP = psM.tile([128, 512], F32, name="misc", tag="misc")
            for _ in range(150):
                nc.tensor.matmul(wupP[0:64, 0:64], wupS[0:1, :], wupS[0:1, :])

            # ---- the pipelined schedule ----
            filler = Filler()
            # chunk 0: emit only the groups attention(0, pr=0) needs up
            # front; the rest drain as filler inside the first windows
            filler.add(make_proj_thunks(0))
            filler.drain(5)

            per_window = {0: 2, 1: 1, 2: 2, 3: 2}
            oproj_sched = {3: [0, 1, 2]}
            # the delayed PV of each chunk's last window carries into the
            # next chunk's first window; it must be flushed only before
            # oproj fillers join the queue (they read the ot tiles the
            # carried norm writes)
            pend = []
            for ch in range(NCH):
                if ch + 1 < NCH:
                    filler.add(make_proj_thunks(ch + 1))
                ops = oproj_sched.get(ch, [])
                if ops and pend:
                    pend.pop()()
                for c in ops:
                    filler.add(make_oproj_thunks(c))
                emit_attention(ch, filler, per_window[ch], pend)
                filler.drain_all()

            if pend:
                pend.pop()()
            filler.add(make_oproj_thunks(NCH - 1, evac_scalar=True))
            filler.drain_all()

    nc.compile()
    return nc


def kernel(x, w_qkv, w_out, b_out):
    if "nc" not in _CACHE:
        _CACHE["nc"] = _build()
    nc = _CACHE["nc"]

    x = np.asarray(x, np.float32)
    w_qkv = np.asarray(w_qkv, np.float32)
    w_out = np.asarray(w_out, np.float32)
    b_out = np.asarray(b_out, np.float32)

    # lower-triangular-inclusive mask for the diagonal 128x128 sub-block:
    # valid where q >= j
    mask = (np.arange(128)[:, None] <= np.arange(128)[None, :]).astype(np.float32)
    in_maps = []
    for c in range(8):
        b, g = c // 2, c % 2
        sl = slice(g * GI, (g + 1) * GI)
        in_maps.append(dict(
            xT=np.ascontiguousarray(x[b].T).astype(ml_dtypes.bfloat16),
            wq=(np.ascontiguousarray(w_qkv[:, sl]) * np.float32(SCALE * LOG2E)).astype(ml_dtypes.bfloat16),
            wk=np.ascontiguousarray(w_qkv[:, 1024 + g * GI:1024 + (g + 1) * GI]).astype(ml_dtypes.bfloat16),
            wv=np.ascontiguousarray(w_qkv[:, 2048 + g * GI:2048 + (g + 1) * GI]).astype(ml_dtypes.bfloat16),
            wo=np.ascontiguousarray(w_out[sl, :]).astype(ml_dtypes.bfloat16),
            msk=mask.astype(ml_dtypes.bfloat16),
            onesf=np.ones((128, 64), ml_dtypes.bfloat16),
        ))
    res = None
    for attempt in range(3):
        try:
            res = run_bass_kernel_spmd(nc, in_maps, core_ids=list(range(8)))
            break
        except Exception:
            if attempt == 2:
                raise
            time.sleep(10)
    _CACHE["res"] = res
    outs = [np.asarray(res.results[c]["out"], np.float32) for c in range(8)]
    full = np.empty((B, N, DIM), np.float32)
    for b in range(B):
        # device outputs are negated (normalization uses -1/den)
        full[b] = b_out[None, :] - outs[2 * b] - outs[2 * b + 1]
    return full


# revision 44
# speedup vs baseline: 1.1708x; 1.0112x over previous
import sys, time
from collections import deque
from functools import partial

sys.path.insert(0, "/opt/trn_rl_repo")
import numpy as np
import ml_dtypes
from concourse import bass, bacc, tile, mybir
from concourse.bass_utils import run_bass_kernel_spmd

F32 = mybir.dt.float32
BF16 = mybir.dt.bfloat16
I32 = mybir.dt.int32

B, N, DIM = 4, 2048, 1024
HEADS, DH = 16, 64
G = 8          # heads per core
GI = G * DH    # 512 = inner width per core
SCALE = DH ** -0.5
LOG2E = 1.4426950408889634
LN2 = 0.6931471805599453
NB = N // 128   # 16 j-blocks
NCH = N // 512  # 4 q-chunks
DT = DIM // 128  # 8 dim tiles
IT = GI // 128   # 4 inner tiles (= head pairs)

PV_DT = BF16

_CACHE = {}


def _build():
    nc = bacc.Bacc(None, target_bir_lowering=False)
    xT = nc.declare_dram_parameter("xT", [DIM, N], BF16, isOutput=False)
    wq = nc.declare_dram_parameter("wq", [DIM, GI], BF16, isOutput=False)
    wk = nc.declare_dram_parameter("wk", [DIM, GI], BF16, isOutput=False)
    wv = nc.declare_dram_parameter("wv", [DIM, GI], BF16, isOutput=False)
    wo = nc.declare_dram_parameter("wo", [GI, DIM], BF16, isOutput=False)
    msk = nc.declare_dram_parameter("msk", [128, 128], PV_DT, isOutput=False)
    onesf = nc.declare_dram_parameter("onesf", [128, 64], BF16, isOutput=False)
    out = nc.declare_dram_parameter("out", [N, DIM], BF16, isOutput=True)

    EXPF = mybir.ActivationFunctionType.Exp
    AL = mybir.AluOpType

    with tile.TileContext(nc) as tc:
        with (
            nc.allow_low_precision(reason="attention P/V in bf16; rel-err gate 2e-2"),
            tc.tile_pool(name="big", bufs=1) as big,
            tc.tile_pool(name="pt", bufs=6) as ptp,
            tc.tile_pool(name="st", bufs=3) as stp,
            tc.tile_pool(name="rc", bufs=3) as rcp,
            tc.tile_pool(name="raw", bufs=4) as rawp,
            tc.tile_pool(name="on", bufs=4) as onp,
            tc.tile_pool(name="psS", bufs=2, space="PSUM") as psS,
            tc.tile_pool(name="psPo", bufs=2, space="PSUM") as psPo,
            tc.tile_pool(name="psM", bufs=2, space="PSUM") as psM,
        ):
            # ---- persistent SBUF ----
            xTall = big.tile([128, DT * N], BF16, name="xTall", tag="xTall")
            wqall = big.tile([128, DT * GI], BF16, name="wqall", tag="wqall")
            wkall = big.tile([128, DT * GI], BF16, name="wkall", tag="wkall")
            wvall = big.tile([128, DT * GI], BF16, name="wvall", tag="wvall")
            woall = big.tile([128, IT * DIM], BF16, name="woall", tag="woall")
            # per-chunk q/k tiles: [ti][ch] -> [128, 512]
            qTc = [[big.tile([128, 512], BF16, name=f"q{i}_{c}", tag=f"q{i}_{c}") for c in range(NCH)]
                   for i in range(IT)]
            kTc = [[big.tile([128, 512], BF16, name=f"k{i}_{c}", tag=f"k{i}_{c}") for c in range(NCH)]
                   for i in range(IT)]
            # v with ones col per head: [128, 8*65] per j-block
            vg = [big.tile([128, G * (DH + 1)], PV_DT, name=f"v{r}", tag=f"v{r}") for r in range(NB)]
            mask = big.tile([128, 128], PV_DT, name="mask", tag="mask")
            ones1 = big.tile([128, 64], BF16, name="ones1", tag="ones1")
            # ot tiles: one generation per chunk (no cross-chunk hazards)
            otg = [[big.tile([128, 512], BF16, name=f"ot{c2}_{i}", tag=f"ot{c2}_{i}") for i in range(IT)]
                   for c2 in range(NCH)]

            # ---- input DMAs: few big strided transfers (issue cost ~600ns
            # each on the initiating engine, so fewer is much better) ----
            def _x_dst(c):
                return xTall[:].rearrange("p (d n) -> p d n", n=N)[:, :, c * 512:(c + 1) * 512]

            def _x_src(c):
                return xT.rearrange("(d p) n -> p d n", p=128)[:, :, c * 512:(c + 1) * 512]

            # x chunk 0 split across the two HW DMA queues so the first
            # k-projection can start as early as possible
            def _x_dst_h(c, h):
                return xTall[:].rearrange("p (d n) -> p d n", n=N)[
                    :, h * 4:(h + 1) * 4, c * 512:(c + 1) * 512]

            def _x_src_h(c, h):
                return xT.rearrange("(d p) n -> p d n", p=128)[
                    :, h * 4:(h + 1) * 4, c * 512:(c + 1) * 512]

            # weights split in half across both HW queues, ordered by first
            # use: wk (k0 proj) -> wq -> wv -> mask; late x chunks go via the
            # gpsimd (SWDGE) queue since they aren't needed until later
            def _w_half(dst, src, h):
                return (dst[:].rearrange("p (d c) -> p d c", c=GI)[:, h * 4:(h + 1) * 4, :],
                        src.rearrange("(d p) c -> p d c", p=128)[:, h * 4:(h + 1) * 4, :])

            nc.sync.dma_start(_x_dst_h(0, 0), _x_src_h(0, 0))
            nc.scalar.dma_start(_x_dst_h(0, 1), _x_src_h(0, 1))
            nc.sync.dma_start(*_w_half(wkall, wk, 0))
            nc.scalar.dma_start(*_w_half(wkall, wk, 1))
            nc.sync.dma_start(*_w_half(wqall, wq, 0))
            nc.scalar.dma_start(*_w_half(wqall, wq, 1))
            nc.sync.dma_start(*_w_half(wvall, wv, 0))
            nc.scalar.dma_start(*_w_half(wvall, wv, 1))
            nc.scalar.dma_start(mask[:], msk[:])
            nc.scalar.dma_start(ones1[:], onesf[:])
            wupS = big.tile([1, 64], BF16, name="wup", tag="wup")
            nc.gpsimd.memset(wupS[:], 1.0)
            for r in range(NB):
                dst = vg[r][:].rearrange("p (h c) -> p h c", c=DH + 1)[:, :, DH:DH + 1]
                nc.gpsimd.memset(dst, 1.0)
            nc.sync.dma_start(_x_dst(1), _x_src(1))
            nc.scalar.dma_start(woall[:].rearrange("p (i c) -> p i c", c=DIM),
                                wo.rearrange("(i p) c -> p i c", p=128))
            # x chunks 2/3 issue later (gated below) so their transfers don't
            # steal HBM bandwidth from the startup-critical wk/wq/wv loads

            # ---- filler machinery ----
            class Filler:
                def __init__(self):
                    self.q = deque()

                def add(self, thunks):
                    self.q.extend(thunks)

                def drain(self, n):
                    for _ in range(n):
                        if not self.q:
                            return
                        self.q.popleft()()

                def drain_all(self):
                    while self.q:
                        self.q.popleft()()

            # ---- projection emission (as thunks) ----
            def proj_mm(kind, g, c, pq, d0, d1):
                for d in range(d0, d1):
                    if kind == "q":
                        nc.tensor.matmul(
                            pq[:],
                            wqall[:, d * GI + g * 128:d * GI + (g + 1) * 128],
                            xTall[:, d * N + c * 512:d * N + (c + 1) * 512],
                            start=(d == 0), stop=(d == DT - 1))
                    elif kind == "k":
                        nc.tensor.matmul(
                            pq[:],
                            wkall[:, d * GI + g * 128:d * GI + (g + 1) * 128],
                            xTall[:, d * N + c * 512:d * N + (c + 1) * 512],
                            start=(d == 0), stop=(d == DT - 1))
                    else:  # v: rows block r = 4c+g
                        r = 4 * c + g
                        nc.tensor.matmul(
                            pq[:],
                            xTall[:, d * N + r * 128:d * N + (r + 1) * 128],
                            wvall[:, d * GI:(d + 1) * GI],
                            start=(d == 0), stop=(d == DT - 1))

            def proj_evac(kind, g, c, pq):
                # proj thunks drain while the scalar engine has idle slack
                # (chunk c drains during attention c-1): evacuating there
                # unblocks psM slot recycling for the PE even when the vector
                # engine is backlogged (masks/norm work)
                cp = nc.scalar.copy
                if kind == "q":
                    cp(qTc[g][c][:], pq[:])
                elif kind == "k":
                    cp(kTc[g][c][:], pq[:])
                else:
                    r = 4 * c + g
                    dst = vg[r][:].rearrange("p (h c2) -> p h c2", c2=DH + 1)[:, :, 0:DH]
                    cp(dst, pq[:].rearrange("p (h c2) -> p h c2", c2=DH))

            def make_proj_thunks(c):
                # one thunk = one full psum group (atomic: alloc+MMs+evac), so
                # no other psM allocation can interleave into a live group
                th = []

                def group(kind, g):
                    pq = psM.tile([128, 512], F32, name="misc", tag="misc")
                    proj_mm(kind, g, c, pq, 0, DT)
                    proj_evac(kind, g, c, pq)

                # c0: all k groups run first (they need only wk, which lands
                # before wq) so the PE stays busy while wq is in flight; v0
                # must drain in window 0, one window before the delayed PV
                # of window 0 reads it
                order = ([("k", 0), ("k", 1), ("k", 2), ("k", 3), ("q", 0),
                          ("v", 0), ("v", 1), ("v", 2), ("q", 1), ("v", 3),
                          ("q", 2), ("q", 3)] if c == 0 else
                         [(kind, g) for kind in ("q", "k", "v")
                          for g in range(IT)])
                for kind, g in order:
                    th.append(partial(group, kind, g))
                return th

            # ---- output projection (as thunks) ----
            def oproj_mm(c, rb, nco, pf, i0, i1):
                ot = otg[c]
                for i in range(i0, i1):
                    nc.tensor.matmul(
                        pf[:], ot[i][:, rb * 128:(rb + 1) * 128],
                        woall[:, i * DIM + nco * 512:i * DIM + (nco + 1) * 512],
                        start=(i == 0), stop=(i == IT - 1))

            def make_oproj_thunks(c, evac_scalar=False):
                th = []

                def group(rb, nco):
                    pf = psM.tile([128, 512], F32, name="misc", tag="misc")
                    oproj_mm(c, rb, nco, pf, 0, IT)
                    so = stp.tile([128, 512], BF16, name="so", tag="so")
                    if evac_scalar:
                        # tail: ACT is idle and DVE-gated psM recycling is
                        # the group-to-group critical path
                        nc.scalar.copy(so[:], pf[:])
                    else:
                        nc.vector.tensor_copy(so[:], pf[:])
                    nc.sync.dma_start(
                        out[c * 512 + rb * 128:c * 512 + (rb + 1) * 128,
                            nco * 512:(nco + 1) * 512], so[:])

                for rb in range(4):
                    for nco in range(2):
                        th.append(partial(group, rb, nco))
                return th

            # ---- attention chunk (triangular skip + delayed PV: window w's
            # PV runs during window w+1, so it never waits on the exp) ----
            def emit_attention(ch, filler, per_window, pend):
                ej = 4 * (ch + 1)
                windows = IT * ej
                rate = (len(filler.q) + 4.0) / windows
                acc = 0.0

                def mk_pv(pr, jb, o, pt, po_pair, first, last):
                    def pv():
                        if first:
                            po_pair.append(psPo.tile([65, 512], F32, name="po", tag="po"))
                            po_pair.append(psPo.tile([65, 512], F32, name="po", tag="po"))
                        for e in range(2):
                            h = 2 * pr + e
                            nc.tensor.matmul(
                                po_pair[e][0:65, o:512],
                                vg[jb][:, h * (DH + 1):(h + 1) * (DH + 1)],
                                pt[:, e * 512 + o:(e + 1) * 512],
                                start=first, stop=last)
                        if last:
                            # evacuate po to SBUF right away so the next
                            # pair's PV isn't blocked on psPo slots
                            raw = rawp.tile([65, 1024], PV_DT, name="raw", tag="raw")
                            if ch == NCH - 1 and pr == IT - 1:
                                # very last pair: this chain is the kernel
                                # tail; split the two evacs across scalar and
                                # vector (parallel PSUM reads, ACT is idle)
                                nc.scalar.copy(raw[:, 0:512], po_pair[0][:])
                            else:
                                nc.vector.tensor_copy(raw[:, 0:512], po_pair[0][:])
                            nc.vector.tensor_copy(raw[:, 512:1024], po_pair[1][:])
                            filler.add([partial(norm_thunk, ch, pr, raw)])
                    return pv

                for pr in range(IT):
                    po_pair = []
                    for jb in range(ej):
                        diag = jb >= ej - 4
                        o = 128 * (jb - (ej - 4)) if diag else 0
                        sp = psS.tile([128, 1024], F32, name="sp", tag="sp")
                        for e in range(2):
                            pb = 64 * e
                            nc.tensor.matmul(
                                sp[:, e * 512 + o:(e + 1) * 512],
                                kTc[pr][jb // 4][pb:pb + 64,
                                                 (jb % 4) * 128:(jb % 4 + 1) * 128],
                                qTc[pr][ch][pb:pb + 64, o:512])
                        pt = ptp.tile([128, 1024], PV_DT, name="pt", tag="pt")
                        if o:
                            spv = sp[:].rearrange("p (e q) -> p e q", q=512)[:, :, o:512]
                            ptv = pt[:].rearrange("p (e q) -> p e q", q=512)[:, :, o:512]
                            nc.scalar.activation(ptv, spv, EXPF, scale=LN2)
                        else:
                            nc.scalar.activation(pt[:], sp[:], EXPF, scale=LN2)
                        if diag:
                            # mask only the 128-wide diagonal sub-block of each half
                            for e in range(2):
                                s = e * 512 + o
                                nc.vector.tensor_mul(pt[:, s:s + 128], pt[:, s:s + 128],
                                                     mask[:, 0:128])
                        if pend:
                            pend.pop()()
                        pend.append(mk_pv(pr, jb, o, pt, po_pair,
                                          jb == 0, jb == ej - 1))
                        acc += rate
                        n = int(acc)
                        acc -= n
                        filler.drain(n)

            def norm_thunk(ch, pr, raw):
                # raw [65, 1024] bf16: cols 0:512 even head (dh rows 0:64, den
                # row 64), cols 512:1024 odd head. Computes NEGATED normalized
                # output (sign folded into the host-side gather).
                I16 = mybir.dt.int16
                d = raw[64:65, :]
                y0 = stp.tile([65, 1024], PV_DT, name="y0", tag="y0")
                nc.vector.tensor_scalar(
                    y0[64:65, :].bitcast(I16), d.bitcast(I16),
                    -1, 0x7EF3, op0=AL.mult, op1=AL.add)
                m = stp.tile([65, 1024], PV_DT, name="m2", tag="m2")
                nc.vector.tensor_mul(m[64:65, :], d, y0[64:65, :])
                rec = rcp.tile([65, 1024], PV_DT, name="rec", tag="rec")
                # rec = (d*y0 - 2) * y0 = -1/d (one Newton step, negated)
                nc.vector.scalar_tensor_tensor(
                    rec[64:65, :], m[64:65, :], 2.0, y0[64:65, :],
                    op0=AL.subtract, op1=AL.mult)
                ot = otg[ch][pr]
                if False:
                    pass
                else:
                    # last chunk: PE broadcast (lowest latency on the tail)
                    prep_e = psM.tile([128, 512], F32, name="misc", tag="misc")
                    nc.tensor.matmul(prep_e[0:64, :], ones1[64:65, 0:64],
                                     rec[64:65, 0:512])
                    prep_o = psM.tile([128, 512], F32, name="misc", tag="misc")
                    nc.tensor.matmul(prep_o[0:64, :], ones1[64:65, 0:64],
                                     rec[64:65, 512:1024])
                    # odd half first: its SBUF->SBUF DMA is the longer pole
                    on_o = onp.tile([64, 512], BF16, name="onorm", tag="onorm")
                    nc.vector.tensor_mul(on_o[:], raw[0:64, 512:1024],
                                         prep_o[0:64, :])
                    nc.sync.dma_start(ot[64:128, :], on_o[:])
                    nc.vector.tensor_mul(ot[0:64, :], raw[0:64, 0:512],
                                         prep_e[0:64, :])

            # ---- HAM warm-up: keep PE busy during the input-DMA wait so the
            # projection phase starts at full clock (reads uninitialized SBUF,
            # results discarded)
            # preload the exp table-set during the input-DMA wait so the first
            # real ACTIVATE doesn't pay the ~2.7us ACT_TABLE_LOAD
            dumA = big.tile([1, 64], PV_DT, name="dumA", tag="dumA")
            nc.scalar.activation(dumA[:], wupS[:], EXPF, scale=LN2)
            wupP = psM.tile([128, 512], F32, name="misc", tag="misc")
            for _ in range(150):
                nc.tensor.matmul(wupP[0:64, 0:64], wupS[0:1, :], wupS[0:1, :])

            # ---- the pipelined schedule ----
            filler = Filler()
            # chunk 0: emit only the groups attention(0, pr=0) needs up
            # front; the rest drain as filler inside the first windows
            filler.add(make_proj_thunks(0))
            filler.drain(5)

            per_window = {0: 2, 1: 1, 2: 2, 3: 2}
            oproj_sched = {3: [0, 1, 2]}
            # the delayed PV of each chunk's last window carries into the
            # next chunk's first window; it must be flushed only before
            # oproj fillers join the queue (they read the ot tiles the
            # carried norm writes)
            pend = []
            for ch in range(NCH):
                if ch + 1 < NCH:
                    filler.add(make_proj_thunks(ch + 1))
                ops = oproj_sched.get(ch, [])
                if ops and pend:
                    pend.pop()()
                for c in ops:
                    filler.add(make_oproj_thunks(c))
                emit_attention(ch, filler, per_window[ch], pend)
                filler.drain_all()
                if ch == 0:
                    # release the x chunk 2/3 loads only once chunk-0
                    # attention output exists: the gpsimd queue gates the
                    # dma issue on this copy, keeping startup HBM bandwidth
                    # for the critical weight loads (chunks 2/3 aren't read
                    # until ~90us in)
                    xgate = big.tile([1, 64], BF16, name="xgate", tag="xgate")
                    nc.gpsimd.tensor_copy(xgate[:], otg[0][0][0:1, 0:64])
                    nc.gpsimd.dma_start(_x_dst(2), _x_src(2))
                    nc.gpsimd.dma_start(_x_dst(3), _x_src(3))

            if pend:
                pend.pop()()
            filler.add(make_oproj_thunks(NCH - 1, evac_scalar=True))
            filler.drain_all()

    nc.compile()
    return nc


def kernel(x, w_qkv, w_out, b_out):
    if "nc" not in _CACHE:
        _CACHE["nc"] = _build()
    nc = _CACHE["nc"]

    x = np.asarray(x, np.float32)
    w_qkv = np.asarray(w_qkv, np.float32)
    w_out = np.asarray(w_out, np.float32)
    b_out = np.asarray(b_out, np.float32)

    # lower-triangular-inclusive mask for the diagonal 128x128 sub-block:
    # valid where q >= j
    mask = (np.arange(128)[:, None] <= np.arange(128)[None, :]).astype(np.float32)
    in_maps = []
    for c in range(8):
        b, g = c // 2, c % 2
        sl = slice(g * GI, (g + 1) * GI)
        in_maps.append(dict(
            xT=np.ascontiguousarray(x[b].T).astype(ml_dtypes.bfloat16),
            wq=(np.ascontiguousarray(w_qkv[:, sl]) * np.float32(SCALE * LOG2E)).astype(ml_dtypes.bfloat16),
            wk=np.ascontiguousarray(w_qkv[:, 1024 + g * GI:1024 + (g + 1) * GI]).astype(ml_dtypes.bfloat16),
            wv=np.ascontiguousarray(w_qkv[:, 2048 + g * GI:2048 + (g + 1) * GI]).astype(ml_dtypes.bfloat16),
            wo=np.ascontiguousarray(w_out[sl, :]).astype(ml_dtypes.bfloat16),
            msk=mask.astype(ml_dtypes.bfloat16),
            onesf=np.ones((128, 64), ml_dtypes.bfloat16),
        ))
    res = None
    for attempt in range(3):
        try:
            res = run_bass_kernel_spmd(nc, in_maps, core_ids=list(range(8)))
            break
        except Exception:
            if attempt == 2:
                raise
            time.sleep(10)
    _CACHE["res"] = res
    outs = [np.asarray(res.results[c]["out"], np.float32) for c in range(8)]
    full = np.empty((B, N, DIM), np.float32)
    for b in range(B):
        # device outputs are negated (normalization uses -1/den)
        full[b] = b_out[None, :] - outs[2 * b] - outs[2 * b + 1]
    return full
